# revision 19
# baseline (speedup 1.0000x reference)
"""Multi-head attention forward on 8 Trainium2 NeuronCores.

Problem: batch=8, seq=1024, d_model=1024, n_heads=16, d_head=64, fp32 ref.

Sharding: data-parallel over batch - core b computes batch element b end to
end (weights replicated, no collectives).

Software-pipelined head-pair schedule: one long loop over head PAIRS in
which the PE rarely idles (keeps the HAM clock gate at K=8/8 = 2.4 GHz; a
phase-split kernel spends ~200us re-throttled at 1.2 GHz because the PE
sits idle behind softmax exp). Deliberately NOT packed to 100% PE
utilization: sustained all-engine saturation trips the chip's P0 power
state, downclocking every engine by 1.2x and losing more than the packing
gains (measured).

  prologue: Q/K projection pair 0 + V heads 0-7 s-tiles 0-4
  slot g:   scores(g) row-tile-paired + exp(g) + PV(g), interleaved with
            QK-proj(g+1) and the remaining V projections
  tail:     output projection, first two s-tiles' pair-0..6 partials
            emitted early to cover pair 7's normalize latency

Per-engine layout tricks:
  - scores^T = K @ Q^T per head has K(contraction)=64: the two heads of a
    pair live on partition halves 0-63 / 64-127, so their score matmuls
    auto-derive PE row-tile positions (0,0)/(64,0) and run CONCURRENTLY in
    the systolic array (2x score throughput).
  - softmax: exp on ScalarE with the 1/8 scale and key-mask folded in as
    activation scale/bias; denominators pop out of the PV matmul via a
    ones-column appended to V (psum row 64).
  - denominator reciprocals run on [128,8] tiles (two per pair) instead of
    [1,512] strips (which cost 4us each on 1 DVE lane).
  - b_Q/b_K are added by DVE tensor_scalar (per-partition scalar AP) during
    the psum->sbuf copy; b_V is folded into b_O on the host
    (out = (Z/den + b_V) W_O + b_O = Z/den W_O + (b_V W_O + b_O)).

Everything is bf16 into the PE with fp32 PSUM accumulation.

This toolchain's walrus encodes at most ONE sync wait per instruction;
_split_multi_waits hoists excess waits onto same-engine EventSemaphore
instructions.
"""

from contextlib import ExitStack

import numpy as np

import concourse.bass as bass
import concourse.tile as tile
from concourse import mybir
from concourse.bass_utils import run_bass_kernel_spmd

S = 1024  # seq
D = 1024  # d_model
H = 16  # heads
E = 64  # d_head
B = 8  # batch == n_cores
P = 128  # partitions
NS = S // P  # 8 s-tiles
ND = D // P  # 8 d-chunks
NG = H // 2  # 8 head pairs

F32 = mybir.dt.float32
BF16 = mybir.dt.bfloat16
AF = mybir.ActivationFunctionType
ADD = mybir.AluOpType.add

MASK_NEG = 60.0  # exp(x - 60) ~ 9e-27: masked keys vanish without inf/nan


def build_program(split_waits=True):
    nc = bass.Bass("TRN2", target_bir_lowering=False, debug=False)

    xt_d = nc.dram_tensor("xt", [P, ND, S], BF16, kind="ExternalInput").ap()
    wq_d = nc.dram_tensor("wq", [P, NG, ND, P], BF16, kind="ExternalInput").ap()
    wk_d = nc.dram_tensor("wk", [P, NG, ND, P], BF16, kind="ExternalInput").ap()
    wv_d = nc.dram_tensor("wv", [P, ND, H * E], BF16, kind="ExternalInput").ap()
    wo_d = nc.dram_tensor("wo", [P, NG, D], BF16, kind="ExternalInput").ap()
    # b_Q / b_K as per-pair partition columns: [:, 0:8]=b_Q, [:, 8:16]=b_K
    bqk_d = nc.dram_tensor("bqk", [P, 2 * NG], F32, kind="ExternalInput").ap()
    bo_d = nc.dram_tensor("bo", [1, D], BF16, kind="ExternalInput").ap()
    mb_d = nc.dram_tensor("mb", [P, NS], F32, kind="ExternalInput").ap()
    out_d = nc.dram_tensor("out", [S, D], F32, kind="ExternalOutput").ap()

    with tile.TileContext(nc) as tc, ExitStack() as ctx:
        g1 = ctx.enter_context(tc.tile_pool(name="g1", bufs=1))

        ones_col = g1.tile([1, P], BF16, tag="ones_col")
        nc.vector.memset(ones_col, 1.0)

        # weights / activations, streamed in fine slices ordered so the
        # prologue's first matmuls start as early as possible
        wq_sb = g1.tile([P, NG, ND, P], BF16, tag="wq_sb")
        wk_sb = g1.tile([P, NG, ND, P], BF16, tag="wk_sb")
        xT = g1.tile([P, ND, S], BF16, tag="xT")
        wv_sb = g1.tile([P, ND, H * E], BF16, tag="wv_sb")
        wo_sb = g1.tile([P, NG, D], BF16, tag="wo_sb")
        mb_sb = g1.tile([P, NS], F32, tag="mb")
        bqk = g1.tile([P, 2 * NG], F32, tag="bqk")
        bo_sb = g1.tile([1, D], BF16, tag="bo")
        bo_bc = g1.tile([P, D], BF16, tag="bo_bc")
        nc.sync.dma_start(out=wq_sb[:, 0], in_=wq_d[:, 0])
        nc.sync.dma_start(out=wk_sb[:, 0], in_=wk_d[:, 0])
        for c in range(ND):
            nc.sync.dma_start(out=xT[:, c], in_=xt_d[:, c])
        nc.sync.dma_start(out=bqk, in_=bqk_d)
        nc.sync.dma_start(out=mb_sb, in_=mb_d)
        for c in range(ND):
            nc.sync.dma_start(out=wv_sb[:, c], in_=wv_d[:, c])
        for g in range(1, NG):
            nc.sync.dma_start(out=wq_sb[:, g], in_=wq_d[:, g])
            nc.sync.dma_start(out=wk_sb[:, g], in_=wk_d[:, g])
        nc.sync.dma_start(out=wo_sb, in_=wo_d)
        nc.sync.dma_start(out=bo_sb, in_=bo_d)
        nc.sync.dma_start(
            out=bo_bc, in_=bo_d.unsqueeze(1).broadcast_to((1, P, D))
        )

        # persistent activations
        qT = g1.tile([P, NG, S], BF16, tag="qT")
        kT = g1.tile([P, NG, S], BF16, tag="kT")
        vb = g1.tile([P, NS, H, E + 1], BF16, tag="vb")
        nc.vector.memset(vb, 1.0)  # pre-sets the softmax-sum ones columns
        zT = g1.tile([P, NG, S], BF16, tag="zT")

        # observer instructions: absorb one new semaphore tick each so later
        # consumers of shared tensors carry at most one wait themselves.
        nc.tensor.ldweights(ones_col)  # DVE tick (memsets)
        nc.tensor.ldweights(xT[:, 0, 0:8])  # xT chunk-0 DMA lane
        act_scrap = g1.tile([P, 1], F32, tag="act_scrap")
        nc.scalar.activation(  # mb DMA lane, observed by ScalarE
            out=act_scrap, in_=mb_sb[:, 0:1], func=AF.Copy
        )

        def qk_group(qp, dst, g, qh, w_sb, bcol):
            for c in range(ND):
                nc.tensor.matmul(
                    out=qp,
                    lhsT=w_sb[:, g, c, :],
                    rhs=xT[:, c, qh * 512 : (qh + 1) * 512],
                    start=(c == 0),
                    stop=(c == ND - 1),
                )
            with nc.allow_low_precision(reason="bf16 q/k with fused bias"):
                nc.vector.tensor_scalar(
                    out=dst[:, g, qh * 512 : (qh + 1) * 512],
                    in0=qp,
                    scalar1=bcol,
                    scalar2=None,
                    op0=ADD,
                )

        def v_group(vp, st, hh):
            for c in range(ND):
                nc.tensor.matmul(
                    out=vp,
                    lhsT=xT[:, c, st * P : (st + 1) * P],
                    rhs=wv_sb[:, c, hh * 512 : (hh + 1) * 512],
                    start=(c == 0),
                    stop=(c == ND - 1),
                )
            nc.vector.tensor_copy(
                out=vb[:, st, hh * 8 : (hh + 1) * 8, 0:E],
                in_=vp.rearrange("p (h e) -> p h e", h=8),
            )

        # one pool scope for prologue + slots + tail: no mid-kernel pool
        # barriers, so the PE never drains at a phase edge
        with (
            tc.tile_pool(name="qpp", bufs=1, space="PSUM") as qpp,
            tc.tile_pool(name="stp", bufs=2, space="PSUM") as stp,
            tc.tile_pool(name="ztp", bufs=3, space="PSUM") as ztp,
            tc.tile_pool(name="ptp", bufs=16) as ptp,
            tc.tile_pool(name="zsbp", bufs=2) as zsbp,
            tc.tile_pool(name="denp", bufs=2) as denp,
            tc.tile_pool(name="rcp", bufs=2) as rcp,
            tc.tile_pool(name="bcp", bufs=4) as bcp,
            tc.tile_pool(name="obp", bufs=2) as obp,
        ):
            # prologue, double-buffered through the (otherwise idle) score
            # psum tiles: just the pair-0 Q/K projections. All V projections
            # ride slot worklists (emitted right before the PV that consumes
            # each s-tile) so nothing waits on the later wv DMA.
            pro = [("qk", qT, 0, 0, wq_sb, bqk[:, 0:1]),
                   ("qk", qT, 0, 1, wq_sb, bqk[:, 0:1]),
                   ("qk", kT, 0, 0, wk_sb, bqk[:, NG : NG + 1]),
                   ("qk", kT, 0, 1, wk_sb, bqk[:, NG : NG + 1])]
            for item in pro:
                t = stp.tile([P, S], F32, tag="st", name="pro")
                qk_group(t[:, 0:512], *item[1:])

            def issue_den(g, zsb, lo):
                den = denp.tile([P, 8], BF16, tag="den", name=f"den{g}{lo}")
                nc.sync.dma_start(out=den, in_=zsb[E : E + 1, lo : lo + 2, :])
                return den

            def finish_norm(g, zsb, lo, den, pos, eng=None):
                # pos: ((half, qh), (half, qh)) for zsb slices lo, lo+1
                rc = rcp.tile([P, 8], BF16, tag="rc", name=f"rc{g}{lo}")
                with nc.allow_low_precision(reason="bf16 softmax denom"):
                    nc.vector.reciprocal(out=rc, in_=den)
                rcs = rcp.tile([1, 2, 512], BF16, tag="rcs", name=f"rcs{g}{lo}")
                nc.sync.dma_start(out=rcs, in_=rc)
                for i, (half, qh) in enumerate(pos):
                    bc = bcp.tile([E, 512], BF16, tag="bc", name=f"bc{g}{lo}{i}")
                    nc.sync.dma_start(
                        out=bc,
                        in_=rcs[:, i, :].unsqueeze(1).broadcast_to((1, E, 512)),
                    )
                    # gpsimd (idle otherwise): keeps the bc-DMA wait out of
                    # the DVE FIFO, which must stay clear for the filler
                    # groups' psum->sbuf copies. Last pair: DVE (faster, and
                    # the DVE is drained by then - tail latency matters).
                    (eng or nc.gpsimd).tensor_mul(
                        zT[half * E : (half + 1) * E, g,
                           qh * 512 : (qh + 1) * 512],
                        zsb[0:E, lo + i, :],
                        bc,
                    )

            for g in range(NG):
                hA, hB = 2 * g, 2 * g + 1
                # filler matmul groups to keep the PE busy while ScalarE exps
                work = []
                vwork = [("v", st, 0) for st in range(NS)] if g == 0 else []
                vwork.reverse()
                if g < NG - 1:
                    for qh in range(2):
                        work.append(("qk", qT, g + 1, qh, wq_sb,
                                     bqk[:, g + 1 : g + 2]))
                    for qh in range(2):
                        work.append(("qk", kT, g + 1, qh, wk_sb,
                                     bqk[:, NG + g + 1 : NG + g + 2]))
                if g < 3:
                    for st in range(3 * g, min(3 * g + 3, NS)):
                        work.append(("v", st, 1))
                if g >= 4:
                    # late slots are filler-light; feed the HAM activity
                    # monitor with tiny K=1 matmuls so the PE never looks
                    # idle long enough to re-throttle to 1.2 GHz
                    work.extend([("warm",)] * (6 if g == NG - 1 else 2))
                work.reverse()

                def emit_one():
                    item = work.pop()
                    qp = qpp.tile([P, 512], F32, tag="qp", name="qp")
                    if item[0] == "qk":
                        qk_group(qp, *item[1:])
                    elif item[0] == "v":
                        v_group(qp, *item[1:])
                    else:
                        for _ in range(3):
                            nc.tensor.matmul(out=qp, lhsT=ones_col,
                                             rhs=bo_sb[:, 0:512],
                                             start=True, stop=True)

                pts = []
                ztA0 = ztB0 = None
                for kt in range(NS):
                    if vwork:  # V for s-tile kt, just ahead of its PV
                        item = vwork.pop()
                        v_group(qpp.tile([P, 512], F32, tag="qp", name="vq"),
                                *item[1:])
                    stA = stp.tile([P, S], F32, tag="st", name=f"stA{g}{kt}")
                    stB = stp.tile([P, S], F32, tag="st", name=f"stB{g}{kt}")
                    for qh in range(2):
                        # pair heads sit on partition halves -> PE row-tiles
                        # (0,0)/(64,0), concurrent in the array
                        nc.tensor.matmul(
                            out=stA[:, qh * 512 : (qh + 1) * 512],
                            lhsT=kT[0:E, g, kt * P : (kt + 1) * P],
                            rhs=qT[0:E, g, qh * 512 : (qh + 1) * 512],
                            start=True,
                            stop=True,
                        )
                        nc.tensor.matmul(
                            out=stB[:, qh * 512 : (qh + 1) * 512],
                            lhsT=kT[E:P, g, kt * P : (kt + 1) * P],
                            rhs=qT[E:P, g, qh * 512 : (qh + 1) * 512],
                            start=True,
                            stop=True,
                        )
                    ptA = ptp.tile([P, S], BF16, tag="pt", name=f"ptA{g}{kt}")
                    ptB = ptp.tile([P, S], BF16, tag="pt", name=f"ptB{g}{kt}")
                    nc.scalar.activation(
                        out=ptA, in_=stA, func=AF.Exp,
                        bias=mb_sb[:, kt : kt + 1], scale=0.125,
                    )
                    nc.scalar.activation(
                        out=ptB, in_=stB, func=AF.Exp,
                        bias=mb_sb[:, kt : kt + 1], scale=0.125,
                    )
                    pts.append((ptA, ptB))
                    if kt == 0:
                        ztA0 = ztp.tile([E + 1, 512], F32, tag="zt", name=f"ztA0{g}")
                        ztB0 = ztp.tile([E + 1, 512], F32, tag="zt", name=f"ztB0{g}")
                    nc.tensor.matmul(
                        out=ztA0, lhsT=vb[:, kt, hA, :], rhs=ptA[:, 0:512],
                        start=(kt == 0), stop=(kt == NS - 1),
                    )
                    nc.tensor.matmul(
                        out=ztB0, lhsT=vb[:, kt, hB, :], rhs=ptB[:, 0:512],
                        start=(kt == 0), stop=(kt == NS - 1),
                    )
                    if work:
                        emit_one()
                while work:
                    emit_one()

                # z rows 0-63 + denominator row 64 go psum->sbuf immediately
                # so psum banks recycle without waiting on the normalize
                zsb = zsbp.tile([E + 1, 4, 512], BF16, tag="zsb", name=f"zsb{g}")
                with nc.allow_low_precision(reason="bf16 z/denominator"):
                    nc.vector.tensor_copy(out=zsb[:, 0, :], in_=ztA0)
                    nc.vector.tensor_copy(out=zsb[:, 1, :], in_=ztB0)
                den0 = issue_den(g, zsb, 0)  # DMA runs during the qh1 passes
                ztA1 = ztp.tile([E + 1, 512], F32, tag="zt", name=f"ztA1{g}")
                for kt in range(NS):
                    nc.tensor.matmul(
                        out=ztA1, lhsT=vb[:, kt, hA, :],
                        rhs=pts[kt][0][:, 512:1024],
                        start=(kt == 0), stop=(kt == NS - 1),
                    )
                ztB1 = ztp.tile([E + 1, 512], F32, tag="zt", name=f"ztB1{g}")
                with nc.allow_low_precision(reason="bf16 z/denominator"):
                    nc.vector.tensor_copy(out=zsb[:, 2, :], in_=ztA1)
                for kt in range(NS):
                    nc.tensor.matmul(
                        out=ztB1, lhsT=vb[:, kt, hB, :],
                        rhs=pts[kt][1][:, 512:1024],
                        start=(kt == 0), stop=(kt == NS - 1),
                    )
                with nc.allow_low_precision(reason="bf16 z/denominator"):
                    nc.vector.tensor_copy(out=zsb[:, 3, :], in_=ztB1)
                den1 = issue_den(g, zsb, 2)
                eng = nc.vector if g == NG - 1 else None
                finish_norm(g, zsb, 0, den0, ((0, 0), (1, 0)), eng)
                finish_norm(g, zsb, 2, den1, ((0, 1), (1, 1)))

            # tail: output projection. The first two s-tiles' pair-0..6
            # partial sums are emitted BEFORE anything touches zT pair 7, so
            # the PE chews on them (and stays HAM-warm) while pair 7's
            # normalize chain (DVE copy -> DMA -> recip -> DMA -> mul) drains.
            def o_group(st, ops, first, last):
                for g in range(first, last + 1):
                    for dh in range(2):
                        nc.tensor.matmul(
                            out=ops[dh],
                            lhsT=zT[:, g, st * P : (st + 1) * P],
                            rhs=wo_sb[:, g, dh * 512 : (dh + 1) * 512],
                            start=(g == 0),
                            stop=(g == NG - 1),
                        )
                if last == NG - 1:
                    ob = obp.tile([P, D], F32, tag="ob", name=f"ob{st}")
                    for dh in range(2):  # b_O rides the psum->sbuf copy
                        nc.vector.tensor_add(
                            ob[:, dh * 512 : (dh + 1) * 512],
                            ops[dh],
                            bo_bc[:, dh * 512 : (dh + 1) * 512],
                        )
                        nc.sync.dma_start(
                            out=out_d[st * P : (st + 1) * P,
                                      dh * 512 : (dh + 1) * 512],
                            in_=ob[:, dh * 512 : (dh + 1) * 512],
                        )

            def o_tiles(st):
                m = st % 4
                if m in (0, 1):  # score tiles: freed by mid-slot-7 exps
                    t = stp.tile([P, S], F32, tag="st", name=f"op{st}")
                    return [t[:, 0:512], t[:, 512:1024]]
                if m == 2:
                    return [ztp.tile([P, 512], F32, tag="zt", name=f"op{st}{i}")
                            for i in range(2)]
                return [ztp.tile([P, 512], F32, tag="zt", name=f"op{st}0"),
                        qpp.tile([P, 512], F32, tag="qp", name=f"op{st}1")]

            opss = [o_tiles(st) for st in range(4)]
            for st in range(4):
                o_group(st, opss[st], 0, NG - 2)
            for st in range(4):
                o_group(st, opss[st], NG - 1, NG - 1)
            for st in range(4, NS):
                o_group(st, o_tiles(st), 0, NG - 1)

    if split_waits:
        _split_multi_waits(nc)
    return nc


def _split_multi_waits(nc):
    """This walrus build encodes at most ONE sync wait per instruction.
    Tile emits more. Hoist excess waits onto same-engine EventSemaphore
    instructions inserted immediately before the offender - engines and
    DGE sequencers execute their streams in order, so this preserves
    semantics exactly."""
    n = 0
    for fn in nc.m.functions:
        for bb in fn.blocks:
            out = []
            for inst in bb.instructions:
                si = getattr(inst, "sync_info", None)
                waits = list(si.on_wait) if si is not None and si.on_wait else []
                if len(waits) > 1:
                    for w in waits[:-1]:
                        n += 1
                        out.append(
                            mybir.InstEventSemaphore(
                                name=f"evw-{n}",
                                engine=inst.engine,
                                sync_info=mybir.SyncInfo(
                                    on_wait=[w], on_update=[]
                                ),
                            )
                        )
                    si.on_wait = [waits[-1]]
                out.append(inst)
            bb.instructions[:] = out


_NC_CACHE = None


def _get_nc():
    global _NC_CACHE
    if _NC_CACHE is None:
        _NC_CACHE = build_program()
    return _NC_CACHE


def _make_in_maps(inputs):
    import ml_dtypes

    bf16 = ml_dtypes.bfloat16
    x = np.asarray(inputs["x"], np.float32)
    mask = np.asarray(inputs["key_attention_mask"])
    wq = np.asarray(inputs["W_Q"], np.float32).astype(bf16)
    wk = np.asarray(inputs["W_K"], np.float32).astype(bf16)
    wv = np.asarray(inputs["W_V"], np.float32).astype(bf16)
    wo = np.asarray(inputs["W_O"], np.float32).astype(bf16)

    def pack_qk(w):  # (H, D, E) -> [p, g, c, (h2 e)]
        return np.ascontiguousarray(
            w.reshape(NG, 2, ND, P, E).transpose(3, 0, 2, 1, 4).reshape(P, NG, ND, P)
        )

    def pack_bcol(b):  # (H, E) -> [(h2 e), g]
        return b.reshape(NG, 2, E).transpose(1, 2, 0).reshape(P, NG)

    bqk = np.concatenate(
        [
            pack_bcol(np.asarray(inputs["b_Q"], np.float32)),
            pack_bcol(np.asarray(inputs["b_K"], np.float32)),
        ],
        axis=1,
    )
    # fold b_V into b_O: out = (Z/den + b_V) W_O + b_O
    bo = np.asarray(inputs["b_O"], np.float64) + np.einsum(
        "he,hed->d",
        np.asarray(inputs["b_V"], np.float64),
        np.asarray(inputs["W_O"], np.float64),
    )
    shared = {
        "wq": pack_qk(wq),
        "wk": pack_qk(wk),
        # (H, D, E) -> [p, c, (h e)]
        "wv": np.ascontiguousarray(
            wv.reshape(H, ND, P, E).transpose(2, 1, 0, 3).reshape(P, ND, H * E)
        ),
        # (H, E, D) -> [(h2 e), g, d]
        "wo": np.ascontiguousarray(
            wo.reshape(NG, 2, E, D).transpose(1, 2, 0, 3).reshape(P, NG, D)
        ),
        "bqk": np.ascontiguousarray(bqk),
        "bo": bo.astype(np.float32).astype(bf16).reshape(1, D),
    }
    in_maps = []
    for b in range(B):
        m = dict(shared)
        xt = x[b].T.astype(bf16)  # (D, S) -> [p, c, s]
        m["xt"] = np.ascontiguousarray(
            xt.reshape(ND, P, S).transpose(1, 0, 2)
        )
        mb = ((mask[b] != 0).astype(np.float32) - 1.0) * MASK_NEG
        m["mb"] = np.ascontiguousarray(mb.reshape(NS, P).T)
        in_maps.append(m)
    return in_maps


def run(inputs, trace=False):
    nc = _get_nc()
    res = run_bass_kernel_spmd(nc, _make_in_maps(inputs), list(range(B)),
                               trace=trace)
    out = np.stack([res.results[b]["out"] for b in range(B)], axis=0)
    return out, res


def kernel(**inputs) -> np.ndarray:
    out, _ = run(inputs, trace=False)
    return out


# revision 20
# speedup vs baseline: 1.0112x; 1.0112x over previous
"""Multi-head attention forward on 8 Trainium2 NeuronCores.

Problem: batch=8, seq=1024, d_model=1024, n_heads=16, d_head=64, fp32 ref.

Sharding: data-parallel over batch - core b computes batch element b end to
end (weights replicated, no collectives).

Software-pipelined head-pair schedule: one long loop over head PAIRS in
which the PE rarely idles (keeps the HAM clock gate at K=8/8 = 2.4 GHz; a
phase-split kernel spends ~200us re-throttled at 1.2 GHz because the PE
sits idle behind softmax exp). Deliberately NOT packed to 100% PE
utilization: sustained all-engine saturation trips the chip's P0 power
state, downclocking every engine by 1.2x and losing more than the packing
gains (measured).

  prologue: Q/K projection pair 0 + V heads 0-7 s-tiles 0-4
  slot g:   scores(g) row-tile-paired + exp(g) + PV(g), interleaved with
            QK-proj(g+1) and the remaining V projections
  tail:     output projection, first two s-tiles' pair-0..6 partials
            emitted early to cover pair 7's normalize latency

Per-engine layout tricks:
  - scores^T = K @ Q^T per head has K(contraction)=64: the two heads of a
    pair live on partition halves 0-63 / 64-127, so their score matmuls
    auto-derive PE row-tile positions (0,0)/(64,0) and run CONCURRENTLY in
    the systolic array (2x score throughput).
  - softmax: exp on ScalarE with the 1/8 scale and key-mask folded in as
    activation scale/bias; denominators pop out of the PV matmul via a
    ones-column appended to V (psum row 64).
  - denominator reciprocals run on [128,8] tiles (two per pair) instead of
    [1,512] strips (which cost 4us each on 1 DVE lane).
  - b_Q/b_K are added by DVE tensor_scalar (per-partition scalar AP) during
    the psum->sbuf copy; b_V is folded into b_O on the host
    (out = (Z/den + b_V) W_O + b_O = Z/den W_O + (b_V W_O + b_O)).

Everything is bf16 into the PE with fp32 PSUM accumulation.

This toolchain's walrus encodes at most ONE sync wait per instruction;
_split_multi_waits hoists excess waits onto same-engine EventSemaphore
instructions.
"""

from contextlib import ExitStack

import numpy as np

import concourse.bass as bass
import concourse.tile as tile
from concourse import mybir
from concourse.bass_utils import run_bass_kernel_spmd

S = 1024  # seq
D = 1024  # d_model
H = 16  # heads
E = 64  # d_head
B = 8  # batch == n_cores
P = 128  # partitions
NS = S // P  # 8 s-tiles
ND = D // P  # 8 d-chunks
NG = H // 2  # 8 head pairs

F32 = mybir.dt.float32
BF16 = mybir.dt.bfloat16
AF = mybir.ActivationFunctionType
ADD = mybir.AluOpType.add

MASK_NEG = 60.0  # exp(x - 60) ~ 9e-27: masked keys vanish without inf/nan


def build_program(split_waits=True):
    nc = bass.Bass("TRN2", target_bir_lowering=False, debug=False)

    xt_d = nc.dram_tensor("xt", [P, ND, S], BF16, kind="ExternalInput").ap()
    wq_d = nc.dram_tensor("wq", [P, NG, ND, P], BF16, kind="ExternalInput").ap()
    wk_d = nc.dram_tensor("wk", [P, NG, ND, P], BF16, kind="ExternalInput").ap()
    wv_d = nc.dram_tensor("wv", [P, ND, H * E], BF16, kind="ExternalInput").ap()
    wo_d = nc.dram_tensor("wo", [P, NG, D], BF16, kind="ExternalInput").ap()
    # b_Q / b_K as per-pair partition columns: [:, 0:8]=b_Q, [:, 8:16]=b_K
    bqk_d = nc.dram_tensor("bqk", [P, 2 * NG], F32, kind="ExternalInput").ap()
    bo_d = nc.dram_tensor("bo", [1, D], BF16, kind="ExternalInput").ap()
    mb_d = nc.dram_tensor("mb", [P, NS], F32, kind="ExternalInput").ap()
    out_d = nc.dram_tensor("out", [S, D], F32, kind="ExternalOutput").ap()

    with tile.TileContext(nc) as tc, ExitStack() as ctx:
        g1 = ctx.enter_context(tc.tile_pool(name="g1", bufs=1))

        ones_col = g1.tile([1, P], BF16, tag="ones_col")
        nc.vector.memset(ones_col, 1.0)

        # weights / activations, streamed in fine slices ordered so the
        # prologue's first matmuls start as early as possible
        wq_sb = g1.tile([P, NG, ND, P], BF16, tag="wq_sb")
        wk_sb = g1.tile([P, NG, ND, P], BF16, tag="wk_sb")
        xT = g1.tile([P, ND, S], BF16, tag="xT")
        wv_sb = g1.tile([P, ND, H * E], BF16, tag="wv_sb")
        wo_sb = g1.tile([P, NG, D], BF16, tag="wo_sb")
        mb_sb = g1.tile([P, NS], F32, tag="mb")
        bqk = g1.tile([P, 2 * NG], F32, tag="bqk")
        bo_sb = g1.tile([1, D], BF16, tag="bo")
        bo_bc = g1.tile([P, D], BF16, tag="bo_bc")
        nc.sync.dma_start(out=wq_sb[:, 0], in_=wq_d[:, 0])
        nc.sync.dma_start(out=wk_sb[:, 0], in_=wk_d[:, 0])
        for c in range(ND):
            nc.sync.dma_start(out=xT[:, c], in_=xt_d[:, c])
        nc.sync.dma_start(out=bqk, in_=bqk_d)
        nc.sync.dma_start(out=mb_sb, in_=mb_d)
        for c in range(ND):
            nc.sync.dma_start(out=wv_sb[:, c], in_=wv_d[:, c])
        for g in range(1, NG):
            nc.sync.dma_start(out=wq_sb[:, g], in_=wq_d[:, g])
            nc.sync.dma_start(out=wk_sb[:, g], in_=wk_d[:, g])
        nc.sync.dma_start(out=wo_sb, in_=wo_d)
        nc.sync.dma_start(out=bo_sb, in_=bo_d)
        nc.sync.dma_start(
            out=bo_bc, in_=bo_d.unsqueeze(1).broadcast_to((1, P, D))
        )

        # persistent activations
        qT = g1.tile([P, NG, S], BF16, tag="qT")
        kT = g1.tile([P, NG, S], BF16, tag="kT")
        vb = g1.tile([P, NS, H, E + 1], BF16, tag="vb")
        nc.vector.memset(vb, 1.0)  # pre-sets the softmax-sum ones columns
        zT = g1.tile([P, NG, S], BF16, tag="zT")

        # observer instructions: absorb one new semaphore tick each so later
        # consumers of shared tensors carry at most one wait themselves.
        nc.tensor.ldweights(ones_col)  # DVE tick (memsets)
        nc.tensor.ldweights(xT[:, 0, 0:8])  # xT chunk-0 DMA lane
        act_scrap = g1.tile([P, 1], F32, tag="act_scrap")
        nc.scalar.activation(  # mb DMA lane, observed by ScalarE
            out=act_scrap, in_=mb_sb[:, 0:1], func=AF.Copy
        )

        def qk_group(qp, dst, g, qh, w_sb, bcol):
            for c in range(ND):
                nc.tensor.matmul(
                    out=qp,
                    lhsT=w_sb[:, g, c, :],
                    rhs=xT[:, c, qh * 512 : (qh + 1) * 512],
                    start=(c == 0),
                    stop=(c == ND - 1),
                )
            with nc.allow_low_precision(reason="bf16 q/k with fused bias"):
                nc.vector.tensor_scalar(
                    out=dst[:, g, qh * 512 : (qh + 1) * 512],
                    in0=qp,
                    scalar1=bcol,
                    scalar2=None,
                    op0=ADD,
                )

        def v_group(vp, st, hh):
            for c in range(ND):
                nc.tensor.matmul(
                    out=vp,
                    lhsT=xT[:, c, st * P : (st + 1) * P],
                    rhs=wv_sb[:, c, hh * 512 : (hh + 1) * 512],
                    start=(c == 0),
                    stop=(c == ND - 1),
                )
            nc.vector.tensor_copy(
                out=vb[:, st, hh * 8 : (hh + 1) * 8, 0:E],
                in_=vp.rearrange("p (h e) -> p h e", h=8),
            )

        # one pool scope for prologue + slots + tail: no mid-kernel pool
        # barriers, so the PE never drains at a phase edge
        with (
            tc.tile_pool(name="qpp", bufs=1, space="PSUM") as qpp,
            tc.tile_pool(name="stp", bufs=2, space="PSUM") as stp,
            tc.tile_pool(name="ztp", bufs=3, space="PSUM") as ztp,
            tc.tile_pool(name="ptp", bufs=16) as ptp,
            tc.tile_pool(name="zsbp", bufs=2) as zsbp,
            tc.tile_pool(name="denp", bufs=2) as denp,
            tc.tile_pool(name="rcp", bufs=2) as rcp,
            tc.tile_pool(name="bcp", bufs=4) as bcp,
            tc.tile_pool(name="obp", bufs=2) as obp,
        ):
            # prologue, double-buffered through the (otherwise idle) score
            # psum tiles: just the pair-0 Q/K projections. All V projections
            # ride slot worklists (emitted right before the PV that consumes
            # each s-tile) so nothing waits on the later wv DMA.
            pro = [("qk", qT, 0, 0, wq_sb, bqk[:, 0:1]),
                   ("qk", qT, 0, 1, wq_sb, bqk[:, 0:1]),
                   ("qk", kT, 0, 0, wk_sb, bqk[:, NG : NG + 1]),
                   ("qk", kT, 0, 1, wk_sb, bqk[:, NG : NG + 1])]
            for item in pro:
                t = stp.tile([P, S], F32, tag="st", name="pro")
                qk_group(t[:, 0:512], *item[1:])

            def issue_den(g, zsb, lo):
                den = denp.tile([P, 8], BF16, tag="den", name=f"den{g}{lo}")
                nc.sync.dma_start(out=den, in_=zsb[E : E + 1, lo : lo + 2, :])
                return den

            def finish_norm(g, zsb, lo, den, pos, eng=None):
                # pos: ((half, qh), (half, qh)) for zsb slices lo, lo+1
                rc = rcp.tile([P, 8], BF16, tag="rc", name=f"rc{g}{lo}")
                with nc.allow_low_precision(reason="bf16 softmax denom"):
                    nc.vector.reciprocal(out=rc, in_=den)
                rcs = rcp.tile([1, 2, 512], BF16, tag="rcs", name=f"rcs{g}{lo}")
                nc.sync.dma_start(out=rcs, in_=rc)
                for i, (half, qh) in enumerate(pos):
                    bc = bcp.tile([E, 512], BF16, tag="bc", name=f"bc{g}{lo}{i}")
                    nc.sync.dma_start(
                        out=bc,
                        in_=rcs[:, i, :].unsqueeze(1).broadcast_to((1, E, 512)),
                    )
                    # gpsimd (idle otherwise): keeps the bc-DMA wait out of
                    # the DVE FIFO, which must stay clear for the filler
                    # groups' psum->sbuf copies. Last pair: DVE (faster, and
                    # the DVE is drained by then - tail latency matters).
                    (eng or nc.gpsimd).tensor_mul(
                        zT[half * E : (half + 1) * E, g,
                           qh * 512 : (qh + 1) * 512],
                        zsb[0:E, lo + i, :],
                        bc,
                    )

            for g in range(NG):
                hA, hB = 2 * g, 2 * g + 1
                # filler matmul groups to keep the PE busy while ScalarE exps
                work = []
                vwork = [("v", st, 0) for st in range(NS)] if g == 0 else []
                vwork.reverse()
                if g < NG - 1:
                    for qh in range(2):
                        work.append(("qk", qT, g + 1, qh, wq_sb,
                                     bqk[:, g + 1 : g + 2]))
                    for qh in range(2):
                        work.append(("qk", kT, g + 1, qh, wk_sb,
                                     bqk[:, NG + g + 1 : NG + g + 2]))
                if g < 3:
                    for st in range(3 * g, min(3 * g + 3, NS)):
                        work.append(("v", st, 1))
                if g >= 4:
                    # late slots are filler-light; feed the HAM activity
                    # monitor with tiny K=1 matmuls so the PE never looks
                    # idle long enough to re-throttle to 1.2 GHz
                    work.extend([("warm",)] * (6 if g == NG - 1 else 2))
                work.reverse()

                def emit_one():
                    item = work.pop()
                    qp = qpp.tile([P, 512], F32, tag="qp", name="qp")
                    if item[0] == "qk":
                        qk_group(qp, *item[1:])
                    elif item[0] == "v":
                        v_group(qp, *item[1:])
                    else:
                        for _ in range(3):
                            nc.tensor.matmul(out=qp, lhsT=ones_col,
                                             rhs=bo_sb[:, 0:512],
                                             start=True, stop=True)

                pts = []
                ztA0 = ztB0 = None
                for kt in range(NS + 1):
                    if kt < NS:
                        if vwork:  # V for s-tile kt, just ahead of its PV
                            item = vwork.pop()
                            v_group(qpp.tile([P, 512], F32, tag="qp",
                                             name="vq"), *item[1:])
                        stA = stp.tile([P, S], F32, tag="st", name=f"stA{g}{kt}")
                        stB = stp.tile([P, S], F32, tag="st", name=f"stB{g}{kt}")
                        for qh in range(2):
                            # pair heads sit on partition halves -> PE
                            # row-tiles (0,0)/(64,0), concurrent in the array
                            nc.tensor.matmul(
                                out=stA[:, qh * 512 : (qh + 1) * 512],
                                lhsT=kT[0:E, g, kt * P : (kt + 1) * P],
                                rhs=qT[0:E, g, qh * 512 : (qh + 1) * 512],
                                start=True,
                                stop=True,
                            )
                            nc.tensor.matmul(
                                out=stB[:, qh * 512 : (qh + 1) * 512],
                                lhsT=kT[E:P, g, kt * P : (kt + 1) * P],
                                rhs=qT[E:P, g, qh * 512 : (qh + 1) * 512],
                                start=True,
                                stop=True,
                            )
                        ptA = ptp.tile([P, S], BF16, tag="pt", name=f"ptA{g}{kt}")
                        ptB = ptp.tile([P, S], BF16, tag="pt", name=f"ptB{g}{kt}")
                        nc.scalar.activation(
                            out=ptA, in_=stA, func=AF.Exp,
                            bias=mb_sb[:, kt : kt + 1], scale=0.125,
                        )
                        nc.scalar.activation(
                            out=ptB, in_=stB, func=AF.Exp,
                            bias=mb_sb[:, kt : kt + 1], scale=0.125,
                        )
                        pts.append((ptA, ptB))
                    if kt > 0:
                        # PV trails the scores/exp stream by one k-tile so it
                        # never waits on a just-finished exp
                        pv = kt - 1
                        if pv == 0:
                            ztA0 = ztp.tile([E + 1, 512], F32, tag="zt",
                                            name=f"ztA0{g}")
                            ztB0 = ztp.tile([E + 1, 512], F32, tag="zt",
                                            name=f"ztB0{g}")
                        nc.tensor.matmul(
                            out=ztA0, lhsT=vb[:, pv, hA, :],
                            rhs=pts[pv][0][:, 0:512],
                            start=(pv == 0), stop=(pv == NS - 1),
                        )
                        nc.tensor.matmul(
                            out=ztB0, lhsT=vb[:, pv, hB, :],
                            rhs=pts[pv][1][:, 0:512],
                            start=(pv == 0), stop=(pv == NS - 1),
                        )
                    if kt < NS and work:
                        emit_one()
                while work:
                    emit_one()

                # z rows 0-63 + denominator row 64 go psum->sbuf immediately
                # so psum banks recycle without waiting on the normalize
                zsb = zsbp.tile([E + 1, 4, 512], BF16, tag="zsb", name=f"zsb{g}")
                with nc.allow_low_precision(reason="bf16 z/denominator"):
                    nc.vector.tensor_copy(out=zsb[:, 0, :], in_=ztA0)
                    nc.vector.tensor_copy(out=zsb[:, 1, :], in_=ztB0)
                den0 = issue_den(g, zsb, 0)  # DMA runs during the qh1 passes
                ztA1 = ztp.tile([E + 1, 512], F32, tag="zt", name=f"ztA1{g}")
                for kt in range(NS):
                    nc.tensor.matmul(
                        out=ztA1, lhsT=vb[:, kt, hA, :],
                        rhs=pts[kt][0][:, 512:1024],
                        start=(kt == 0), stop=(kt == NS - 1),
                    )
                ztB1 = ztp.tile([E + 1, 512], F32, tag="zt", name=f"ztB1{g}")
                with nc.allow_low_precision(reason="bf16 z/denominator"):
                    nc.vector.tensor_copy(out=zsb[:, 2, :], in_=ztA1)
                for kt in range(NS):
                    nc.tensor.matmul(
                        out=ztB1, lhsT=vb[:, kt, hB, :],
                        rhs=pts[kt][1][:, 512:1024],
                        start=(kt == 0), stop=(kt == NS - 1),
                    )
                with nc.allow_low_precision(reason="bf16 z/denominator"):
                    nc.vector.tensor_copy(out=zsb[:, 3, :], in_=ztB1)
                den1 = issue_den(g, zsb, 2)
                eng = nc.vector if g == NG - 1 else None
                finish_norm(g, zsb, 0, den0, ((0, 0), (1, 0)), eng)
                finish_norm(g, zsb, 2, den1, ((0, 1), (1, 1)))

            # tail: output projection. The first two s-tiles' pair-0..6
            # partial sums are emitted BEFORE anything touches zT pair 7, so
            # the PE chews on them (and stays HAM-warm) while pair 7's
            # normalize chain (DVE copy -> DMA -> recip -> DMA -> mul) drains.
            def o_group(st, ops, first, last):
                for g in range(first, last + 1):
                    for dh in range(2):
                        nc.tensor.matmul(
                            out=ops[dh],
                            lhsT=zT[:, g, st * P : (st + 1) * P],
                            rhs=wo_sb[:, g, dh * 512 : (dh + 1) * 512],
                            start=(g == 0),
                            stop=(g == NG - 1),
                        )
                if last == NG - 1:
                    ob = obp.tile([P, D], F32, tag="ob", name=f"ob{st}")
                    for dh in range(2):  # b_O rides the psum->sbuf copy
                        nc.vector.tensor_add(
                            ob[:, dh * 512 : (dh + 1) * 512],
                            ops[dh],
                            bo_bc[:, dh * 512 : (dh + 1) * 512],
                        )
                        nc.sync.dma_start(
                            out=out_d[st * P : (st + 1) * P,
                                      dh * 512 : (dh + 1) * 512],
                            in_=ob[:, dh * 512 : (dh + 1) * 512],
                        )

            def o_tiles(st):
                m = st % 4
                if m in (0, 1):  # score tiles: freed by mid-slot-7 exps
                    t = stp.tile([P, S], F32, tag="st", name=f"op{st}")
                    return [t[:, 0:512], t[:, 512:1024]]
                if m == 2:
                    return [ztp.tile([P, 512], F32, tag="zt", name=f"op{st}{i}")
                            for i in range(2)]
                return [ztp.tile([P, 512], F32, tag="zt", name=f"op{st}0"),
                        qpp.tile([P, 512], F32, tag="qp", name=f"op{st}1")]

            opss = [o_tiles(st) for st in range(4)]
            for st in range(4):
                o_group(st, opss[st], 0, NG - 2)
            for st in range(4):
                o_group(st, opss[st], NG - 1, NG - 1)
            for st in range(4, NS):
                o_group(st, o_tiles(st), 0, NG - 1)

    if split_waits:
        _split_multi_waits(nc)
    return nc


def _split_multi_waits(nc):
    """This walrus build encodes at most ONE sync wait per instruction.
    Tile emits more. Hoist excess waits onto same-engine EventSemaphore
    instructions inserted immediately before the offender - engines and
    DGE sequencers execute their streams in order, so this preserves
    semantics exactly."""
    n = 0
    for fn in nc.m.functions:
        for bb in fn.blocks:
            out = []
            for inst in bb.instructions:
                si = getattr(inst, "sync_info", None)
                waits = list(si.on_wait) if si is not None and si.on_wait else []
                if len(waits) > 1:
                    for w in waits[:-1]:
                        n += 1
                        out.append(
                            mybir.InstEventSemaphore(
                                name=f"evw-{n}",
                                engine=inst.engine,
                                sync_info=mybir.SyncInfo(
                                    on_wait=[w], on_update=[]
                                ),
                            )
                        )
                    si.on_wait = [waits[-1]]
                out.append(inst)
            bb.instructions[:] = out


_NC_CACHE = None


def _get_nc():
    global _NC_CACHE
    if _NC_CACHE is None:
        _NC_CACHE = build_program()
    return _NC_CACHE


def _make_in_maps(inputs):
    import ml_dtypes

    bf16 = ml_dtypes.bfloat16
    x = np.asarray(inputs["x"], np.float32)
    mask = np.asarray(inputs["key_attention_mask"])
    wq = np.asarray(inputs["W_Q"], np.float32).astype(bf16)
    wk = np.asarray(inputs["W_K"], np.float32).astype(bf16)
    wv = np.asarray(inputs["W_V"], np.float32).astype(bf16)
    wo = np.asarray(inputs["W_O"], np.float32).astype(bf16)

    def pack_qk(w):  # (H, D, E) -> [p, g, c, (h2 e)]
        return np.ascontiguousarray(
            w.reshape(NG, 2, ND, P, E).transpose(3, 0, 2, 1, 4).reshape(P, NG, ND, P)
        )

    def pack_bcol(b):  # (H, E) -> [(h2 e), g]
        return b.reshape(NG, 2, E).transpose(1, 2, 0).reshape(P, NG)

    bqk = np.concatenate(
        [
            pack_bcol(np.asarray(inputs["b_Q"], np.float32)),
            pack_bcol(np.asarray(inputs["b_K"], np.float32)),
        ],
        axis=1,
    )
    # fold b_V into b_O: out = (Z/den + b_V) W_O + b_O
    bo = np.asarray(inputs["b_O"], np.float64) + np.einsum(
        "he,hed->d",
        np.asarray(inputs["b_V"], np.float64),
        np.asarray(inputs["W_O"], np.float64),
    )
    shared = {
        "wq": pack_qk(wq),
        "wk": pack_qk(wk),
        # (H, D, E) -> [p, c, (h e)]
        "wv": np.ascontiguousarray(
            wv.reshape(H, ND, P, E).transpose(2, 1, 0, 3).reshape(P, ND, H * E)
        ),
        # (H, E, D) -> [(h2 e), g, d]
        "wo": np.ascontiguousarray(
            wo.reshape(NG, 2, E, D).transpose(1, 2, 0, 3).reshape(P, NG, D)
        ),
        "bqk": np.ascontiguousarray(bqk),
        "bo": bo.astype(np.float32).astype(bf16).reshape(1, D),
    }
    in_maps = []
    for b in range(B):
        m = dict(shared)
        xt = x[b].T.astype(bf16)  # (D, S) -> [p, c, s]
        m["xt"] = np.ascontiguousarray(
            xt.reshape(ND, P, S).transpose(1, 0, 2)
        )
        mb = ((mask[b] != 0).astype(np.float32) - 1.0) * MASK_NEG
        m["mb"] = np.ascontiguousarray(mb.reshape(NS, P).T)
        in_maps.append(m)
    return in_maps


def run(inputs, trace=False):
    nc = _get_nc()
    res = run_bass_kernel_spmd(nc, _make_in_maps(inputs), list(range(B)),
                               trace=trace)
    out = np.stack([res.results[b]["out"] for b in range(B)], axis=0)
    return out, res


def kernel(**inputs) -> np.ndarray:
    out, _ = run(inputs, trace=False)
    return out
